# revision 1
# baseline (speedup 1.0000x reference)
"""Multi-head attention (B=4, L=1024, D=1024, H=16, DH=64) on 8 TRN2 NeuronCores.

Sharding: data-parallel over batch (4) x tensor-parallel over heads (2).
Core c = 2*b + t computes, for batch b, heads [t*8, (t+1)*8):
    QT = Wq_t^T X^T, KT = Wk_t^T X^T, V = Y Wv_t        (all bf16 matmuls)
    per head: S^T = K_h Q_h^T; P^T = exp(S^T/8);
              [ctx^T; rowsum] = Vaug_h^T P^T;  ctxn = ctx / rowsum
    O_partial = ctxn^T Wo_t                              (f32, two dt-halves)
Host pre-transposes X/Y, casts to bf16, and sums the four f32 partials
(2 tensor-parallel cores x 2 dt-halves) per batch.

Engines execute their compiled instruction streams in order, so the emission
order is a hand-software-pipelined schedule: every ST (scores) step, whose exp
drain on the scalar engine is slower than the matmuls, is followed by an
independent fill chain (V projection, next d-tile QT/KT, an earlier head's
ctx, or an out-projection partial) so the tensor engine never waits for the
scalar engine to free an ST PSUM tile.

Perf notes (vs the first working version):
  - Input DMA configs are spread across four sequencers (SP/Pool/DVE/ACT);
    a single SP rail configures queues at ~0.6us each, serializing the
    input rollout and starving the PE for the first ~15us.
  - The ones-blocks of Vaug are memset with one strided op (half the data).
  - The first QT/KT drains go to the scalar engine (idle before the exps).
  - Tail: ctx tiles for the last head pair live in the (by then idle) wide
    ST PSUM pool so the out-projection chains get the full 4-slot acc pool;
    tail drains alternate scalar/vector; chain order puts both ic0 ctx
    chains first so their normalize DMA round-trips hide under ic1's PE
    work. Keeping the PE stream dense also holds it at the 2.4GHz p-state
    (it drops to 1.2GHz within ~100ns of going idle).
"""

import numpy as np
import ml_dtypes

import concourse.tile as tile
import concourse.mybir as mybir
from concourse import bacc
from concourse.bass_utils import run_bass_kernel_spmd

B, L, D, U, H = 4, 1024, 1024, 1024, 16
DH = U // H          # 64 head dim
TP = 2               # tensor-parallel ways (heads)
DL = U // TP         # 512 local units
HL = H // TP         # 8 local heads
P = 128              # partitions
NI = 512             # matmul free-dim chunk (one PSUM bank of f32)
CC = D // P          # 8 contraction chunks for projections
DT = DL // P         # 4 local d-tiles
IT = L // P          # 8 i/j tiles
NIC = L // NI        # 2 free chunks of 512
N_CORES = 8

BF16 = mybir.dt.bfloat16
F32 = mybir.dt.float32


def _build_kernel():
    nc = bacc.Bacc(
        "TRN2", target_bir_lowering=False, debug=False, num_devices=N_CORES
    )
    xt = nc.dram_tensor("xt", [D, L], BF16, kind="ExternalInput").ap()
    yt = nc.dram_tensor("yt", [D, L], BF16, kind="ExternalInput").ap()
    wq = nc.dram_tensor("wq", [D, DL], BF16, kind="ExternalInput").ap()
    wk = nc.dram_tensor("wk", [D, DL], BF16, kind="ExternalInput").ap()
    wv = nc.dram_tensor("wv", [D, DL], BF16, kind="ExternalInput").ap()
    wo = nc.dram_tensor("wo", [DL, U], BF16, kind="ExternalInput").ap()
    out_a = nc.dram_tensor("out_a", [L, U], BF16, kind="ExternalOutput").ap()
    out_c = nc.dram_tensor("out_c", [L, U], BF16, kind="ExternalOutput").ap()

    with tile.TileContext(nc) as tc:
        _mha_body(tc, out_a, out_c, xt, yt, wq, wk, wv, wo)

    nc.compile()
    return nc


def _mha_body(tc, out_a, out_c, xt, yt, wq, wk, wv, wo, dbg=None):
    nc = tc.nc
    from contextlib import ExitStack

    with ExitStack() as ctx:
        persist = ctx.enter_context(tc.tile_pool(name="persist", bufs=1))
        pt_pool = ctx.enter_context(tc.tile_pool(name="pt", bufs=4))
        # ST tiles: [P, 1024] f32 = 2 banks each
        ps_wide = ctx.enter_context(tc.tile_pool(name="ps_wide", bufs=2, space="PSUM"))
        # single-bank accumulators (projections, V, ctx, out-proj)
        ps_acc = ctx.enter_context(tc.tile_pool(name="ps_acc", bufs=4, space="PSUM"))
        small = ctx.enter_context(tc.tile_pool(name="small", bufs=4))

        # persistent SBUF tensors
        xt_sb = persist.tile([P, CC, L], BF16, tag="xt")
        yt_sb = persist.tile([P, CC, L], BF16, tag="yt")
        wq_sb = persist.tile([P, CC, DL], BF16, tag="wq")
        wk_sb = persist.tile([P, CC, DL], BF16, tag="wk")
        wv_sb = persist.tile([P, CC, DL], BF16, tag="wv")
        wo_sb = persist.tile([P, DT, U], BF16, tag="wo")
        qt_sb = persist.tile([P, DT, L], BF16, tag="qt")
        kt_sb = persist.tile([P, DT, L], BF16, tag="kt")
        # Vaug: per j-chunk, per head a 128-col block; even h: [V_h | ones],
        # odd h: [ones | V_h] (ctx^T lands on the head's own cx partitions)
        va_sb = persist.tile([P, IT, HL * P], BF16, tag="va")
        cx_sb = persist.tile([P, DT, L], BF16, tag="cx")

        # Input DMA rollout. Each dma_start costs ~0.6us of CONFIG time on
        # its issuing sequencer, so 33 configs on one rail would serialize
        # the rollout over ~20us. Spread them over the three DMA-capable
        # sequencers (SP / Pool / ACT), first-needed tensors first per rail:
        #   SP:   xt (qt chains), then wv (pair-0 fills), wo (late)
        #   Pool: wq (qt chains), then wk (kt chains; no other early work)
        #   ACT:  yt (kt chains; idle until the first exp)
        wq_r = wq.rearrange("(cc p) d -> p cc d", p=P)
        wk_r = wk.rearrange("(cc p) d -> p cc d", p=P)
        wv_r = wv.rearrange("(cc p) d -> p cc d", p=P)
        xt_r = xt.rearrange("(cc p) i -> p cc i", p=P)
        yt_r = yt.rearrange("(cc p) i -> p cc i", p=P)
        for cc in range(CC):
            nc.sync.dma_start(out=xt_sb[:, cc], in_=xt_r[:, cc])
            nc.gpsimd.dma_start(out=wq_sb[:, cc], in_=wq_r[:, cc])
            nc.gpsimd.dma_start(out=wk_sb[:, cc], in_=wk_r[:, cc])
            nc.scalar.dma_start(out=yt_sb[:, cc], in_=yt_r[:, cc])
        for cc in range(CC):
            nc.sync.dma_start(out=wv_sb[:, cc], in_=wv_r[:, cc])
        nc.sync.dma_start(out=wo_sb[:], in_=wo.rearrange("(dt p) o -> p dt o", p=P))

        # ones-blocks of Vaug: columns [64,192) mod 256 of each j-chunk
        # (even heads keep V in the low half, odd heads in the high half).
        # One strided memset over half the tensor; the V halves are written
        # by the v_chain drains.
        va_ones = va_sb.rearrange("p it (q s) -> p it q s", s=2 * P)
        nc.vector.memset(va_ones[:, :, :, DH : DH + P], 1.0)

        scale = DH**-0.5

        # ---- chain emitters (each a short burst of independent PE work) ----

        def proj_chain(w_sb, t_sb, rhs_sb, dt, ic, copy_eng="vector"):
            ps = ps_acc.tile([P, NI], F32, tag="acc")
            for cc in range(CC):
                nc.tensor.matmul(
                    ps[:],
                    w_sb[:, cc, dt * P : (dt + 1) * P],
                    rhs_sb[:, cc, ic * NI : (ic + 1) * NI],
                    start=(cc == 0),
                    stop=(cc == CC - 1),
                )
            dst = t_sb[:, dt, ic * NI : (ic + 1) * NI]
            if copy_eng == "vector":
                nc.vector.tensor_copy(dst, ps[:])
            else:
                nc.scalar.copy(dst, ps[:])

        def v_chain(jt):
            ps = ps_acc.tile([P, NI], F32, tag="acc")
            for cc in range(CC):
                nc.tensor.matmul(
                    ps[:],
                    yt_sb[:, cc, jt * P : (jt + 1) * P],
                    wv_sb[:, cc, :],
                    start=(cc == 0),
                    stop=(cc == CC - 1),
                )
            va_blk = va_sb[:, jt].rearrange("p (h s) -> p h s", s=P)
            ps_blk = ps.rearrange("p (h s) -> p h s", s=DH)
            nc.vector.tensor_copy(va_blk[:, 0::2, 0:DH], ps_blk[:, 0::2, :])
            nc.vector.tensor_copy(va_blk[:, 1::2, DH:P], ps_blk[:, 1::2, :])

        # Deferred finishers: the normalize crosses engines (DVE -> gpsimd
        # partition_broadcast -> DVE); emitting the post-broadcast DVE ops
        # immediately would stall the in-order DVE stream (and the PSUM-
        # releasing copies queued behind it) on the gpsimd semaphore.
        # Instead each ctx chain queues them and the next fill slot flushes.
        deferred = []

        def flush_deferred():
            while deferred:
                deferred.pop(0)()

        def ctx_chain(h, ptile, ic):
            dt, r0 = divmod(h * DH, P)
            ct = ps_acc.tile([P, NI], F32, tag="acc")
            cts = ct[:]
            for jt in range(IT):
                nc.tensor.matmul(
                    cts,
                    va_sb[:, jt, h * P : (h + 1) * P],
                    ptile[:, jt, ic * NI : (ic + 1) * NI],
                    start=(jt == 0),
                    stop=(jt == IT - 1),
                )
            # The 64 rowsum rows of ct are identical copies (each ones-column
            # of Vaug reproduces the row sum), so a gpsimd partition
            # broadcast of a single row moves the rowsum to the partitions
            # the ctx rows live on — no DMA round trip. The custom DVE
            # reciprocal only works at base partition 0.
            rc = small.tile([P, NI], F32, tag="rc")
            if r0 == 0:
                # ctx in rows 0:DH, rowsum copies in rows DH:P. The gpsimd
                # broadcast source must sit at partition 0 (Q7 core 0 owns
                # partitions 0:16 and does the read), so this orientation
                # has to move the rowsum down with a SBUF->SBUF DMA.
                rs = small.tile([P, NI], F32, tag="rs")
                nc.vector.tensor_copy(rs[DH:P, :], cts[DH:P, :])
                nc.gpsimd.dma_start(out=rs[0:DH, :], in_=rs[DH:P, :])

                def fin():
                    nc.vector.reciprocal_approx_fast(rc[0:DH, :], rs[0:DH, :])
                    nc.vector.tensor_mul(
                        cx_sb[0:DH, dt, ic * NI : (ic + 1) * NI],
                        cts[0:DH, :],
                        rc[0:DH, :],
                    )
            else:
                # rowsum copies in rows 0:DH, ctx in rows DH:P: reciprocal
                # of a single row at base 0 (all DH rowsum rows are
                # identical), then gpsimd partition-broadcast (the Q7 impl
                # reads the source on core 0 and write-masks partitions
                # [0, channels) absolutely, so broadcast all 128 rows).
                nc.vector.reciprocal_approx_fast(rc[0:1, :], cts[0:1, :])
                nc.gpsimd.partition_broadcast(rc[0:P, :], rc[0:1, :])

                def fin():
                    nc.vector.tensor_mul(
                        cx_sb[DH:P, dt, ic * NI : (ic + 1) * NI],
                        cts[DH:P, :],
                        rc[DH:P, :],
                    )

            deferred.append(fin)

        def po_chain(it, oc, dts, out_ap, copy_eng="vector", po=None, dma_eng=None):
            # out-projection partial over the given d-tiles
            if po is None:
                po = ps_acc.tile([P, NI], F32, tag="acc")
            for k, dt in enumerate(dts):
                nc.tensor.matmul(
                    po[:],
                    cx_sb[:, dt, it * P : (it + 1) * P],
                    wo_sb[:, dt, oc * NI : (oc + 1) * NI],
                    start=(k == 0),
                    stop=(k == len(dts) - 1),
                )
            o_st = small.tile([P, NI], BF16, tag="ost")
            if copy_eng == "vector":
                nc.vector.tensor_copy(o_st[:], po[:])
            else:
                # scalar engine is idle once the exp stream has drained
                nc.scalar.copy(o_st[:], po[:])
            out_r = out_ap.rearrange("(it p) o -> it p o", p=P)
            dma_eng = dma_eng or nc.sync
            dma_eng.dma_start(
                out=out_r[it, :, oc * NI : (oc + 1) * NI], in_=o_st[:]
            )

        # ---- ST + exp for a head pair, fill chains between steps ----

        def st_pair(hp, fills):
            dt = hp
            ptiles = []
            for h_off in range(2):
                pt_tile = pt_pool.tile([P, IT, L], BF16, tag="pt")
                ptiles.append(pt_tile)
            fills = list(fills)
            for jt in range(IT):
                sts = []
                for h_off in range(2):
                    r0 = DH * h_off
                    st = ps_wide.tile([P, 2 * NI], F32, tag="wide")
                    sts.append(st)
                    for ic in range(NIC):
                        nc.tensor.matmul(
                            st[:, ic * NI : (ic + 1) * NI],
                            kt_sb[r0 : r0 + DH, dt, jt * P : (jt + 1) * P],
                            qt_sb[r0 : r0 + DH, dt, ic * NI : (ic + 1) * NI],
                            start=True,
                            stop=True,
                        )
                for h_off in range(2):
                    nc.scalar.activation(
                        ptiles[h_off][:, jt, :],
                        sts[h_off][:],
                        mybir.ActivationFunctionType.Exp,
                        scale=scale,
                    )
                if jt < len(fills):
                    pending = list(deferred)
                    deferred.clear()
                    for f in fills[jt]:
                        f()
                    for f in pending:
                        f()
            return ptiles

        # ---- schedule ----
        mk = lambda f, *a: (lambda: f(*a))

        # first QT/KT drains on the scalar engine: the ACT stream is idle
        # until the first exp, and this keeps the DVE free for the va
        # memset + early v drains
        for ic in range(NIC):
            proj_chain(wq_sb, qt_sb, xt_sb, 0, ic, copy_eng="scalar")
        for ic in range(NIC):
            proj_chain(wk_sb, kt_sb, yt_sb, 0, ic, copy_eng="scalar")

        # pair 0: fill with the 8 V chains
        pt0 = st_pair(0, [[mk(v_chain, jt)] for jt in range(IT)])

        if dbg is not None:
            nc.sync.dma_start(out=dbg[3][0], in_=pt0[0][:])
            nc.sync.dma_start(out=dbg[3][1], in_=pt0[1][:])

        # QT1/KT1 ahead of pair 1 (also covers pair-0 exp tail)
        for ic in range(NIC):
            proj_chain(wq_sb, qt_sb, xt_sb, 1, ic)
        for ic in range(NIC):
            proj_chain(wk_sb, kt_sb, yt_sb, 1, ic)

        # pair 1: fill with ctx of heads 0/1 and QT2/KT2, interleaved so
        # ctx-chain PSUM tiles (whose release waits on the cross-engine
        # normalize) never claim more than every other acc slot
        pt1 = st_pair(
            1,
            [
                [mk(ctx_chain, 0, pt0[0], 0)],
                [mk(proj_chain, wq_sb, qt_sb, xt_sb, 2, 0)],
                [mk(ctx_chain, 0, pt0[0], 1)],
                [mk(proj_chain, wq_sb, qt_sb, xt_sb, 2, 1)],
                [mk(ctx_chain, 1, pt0[1], 0)],
                [mk(proj_chain, wk_sb, kt_sb, yt_sb, 2, 0)],
                [mk(ctx_chain, 1, pt0[1], 1)],
                [mk(proj_chain, wk_sb, kt_sb, yt_sb, 2, 1)],
            ],
        )

        # pair 2: fill with ctx of heads 2/3 and QT3/KT3
        pt2 = st_pair(
            2,
            [
                [mk(ctx_chain, 2, pt1[0], 0)],
                [mk(proj_chain, wq_sb, qt_sb, xt_sb, 3, 0)],
                [mk(ctx_chain, 2, pt1[0], 1)],
                [mk(proj_chain, wq_sb, qt_sb, xt_sb, 3, 1)],
                [mk(ctx_chain, 3, pt1[1], 0)],
                [mk(proj_chain, wk_sb, kt_sb, yt_sb, 3, 0)],
                [mk(ctx_chain, 3, pt1[1], 1)],
                [mk(proj_chain, wk_sb, kt_sb, yt_sb, 3, 1)],
            ],
        )

        # pair 3: fill with ctx of heads 4/5, then out-proj partial A over
        # dt 0..2 (heads 0..5 — heads 4/5 finish in this phase's early slots,
        # so the late slots can already drain 3/4 of the out-projection)
        poA = [
            mk(po_chain, it, oc, (0, 1, 2), out_a)
            for it in range(IT)
            for oc in range(NIC)
        ]
        # slot 7 keeps only a small fill: anything more delays the critical
        # tail ctx chains (the exp it must cover is only ~1.3us); the two
        # remaining poA chains instead plug the tail's own idle points
        pt3 = st_pair(
            3,
            [
                [mk(ctx_chain, 4, pt2[0], 0)],
                [mk(ctx_chain, 4, pt2[0], 1)],
                [mk(ctx_chain, 5, pt2[1], 0)],
                [mk(ctx_chain, 5, pt2[1], 1)],
                poA[0:4],
                poA[4:8],
                poA[8:12],
                poA[12:15],
            ],
        )

        # tail: ctx of heads 6/7 — each (head, ic) chain gets its own acc
        # tile (a shared tile serializes: the next chain's start-write waits
        # for the previous chain's pending normalize reads). ic0 chains
        # first so their normalize round-trips hide under ic1's matmuls.
        # The dt-3 out-proj chains then rotate through halves of the (by
        # now idle) wide ST tiles so the acc slots held by pending ctx
        # normalizes never gate them; drains alternate scalar/vector.
        # Tail ctx ordering: ic0 pair first (their finishers gate the first
        # out-proj chains), h6 (DMA normalize) before h7 (broadcast). The
        # ic0 finishers flush after the third chain's emission — by then
        # h6ic0's DMA round trip has completed, so nothing in the in-order
        # DVE stream blocks.
        ctx_chain(6, pt3[0], 0)
        ctx_chain(7, pt3[1], 0)
        ctx_chain(6, pt3[0], 1)
        flush_deferred()  # fins for h6ic0 and h7ic0
        ctx_chain(7, pt3[1], 1)
        # the last two out_a chains fill the window where the DVE finishers
        # for the ic0 half are still completing
        for f in poA[14:16]:
            f()
        flush_deferred()  # fins for h6ic1 and h7ic1

        def po_tile_gen():
            # [wide, wide, acc, acc] repeating: the first acc slots the po
            # chains reuse are the ic0 ctx tiles (normalized early); the ic1
            # ctx tiles only come up for reuse once their finishers have
            # run. Wide tiles are used whole (half-sharing serializes on the
            # tile-granular write-after-read hazard).
            while True:
                pw = ps_wide.tile([P, 2 * NI], F32, tag="wide", name="po_w")
                yield pw[:, 0:NI]
                pw = ps_wide.tile([P, 2 * NI], F32, tag="wide", name="po_w")
                yield pw[:, 0:NI]
                yield ps_acc.tile([P, NI], F32, tag="acc", name="po_a")
                yield ps_acc.tile([P, NI], F32, tag="acc", name="po_a")

        po_tiles = po_tile_gen()
        # it-blocks 0..3 only read the ic0-half of cx dt3, whose normalizes
        # are already flushed — emit them before the last (ic1) finishers
        # so their DMA round-trips hide behind real work
        for it in range(IT // 2):
            for oc in range(NIC):
                po_chain(it, oc, (3,), out_c,
                         copy_eng=("scalar", "vector")[oc], po=next(po_tiles))
        for it in range(IT // 2, IT):
            for oc in range(NIC):
                po_chain(it, oc, (3,), out_c,
                         copy_eng=("scalar", "vector")[oc], po=next(po_tiles))

        if dbg is not None:
            nc.sync.dma_start(out=dbg[0][:], in_=qt_sb[:])
            nc.sync.dma_start(out=dbg[1][:], in_=kt_sb[:])
            nc.sync.dma_start(out=dbg[2][:], in_=va_sb[:])
            nc.sync.dma_start(out=dbg[4][:], in_=cx_sb[:])


_NC_CACHE = None


def _get_nc():
    global _NC_CACHE
    if _NC_CACHE is None:
        _NC_CACHE = _build_kernel()
    return _NC_CACHE


def kernel(x, y, Wq, Wk, Wv, Wo, _trace=False):
    bf = ml_dtypes.bfloat16
    x = np.asarray(x, np.float32)
    y = np.asarray(y, np.float32)
    xtb = [np.ascontiguousarray(np.asarray(x[b]).T).astype(bf) for b in range(B)]
    ytb = [np.ascontiguousarray(np.asarray(y[b]).T).astype(bf) for b in range(B)]
    wqs = [np.ascontiguousarray(np.asarray(Wq)[:, t * DL : (t + 1) * DL]).astype(bf) for t in range(TP)]
    wks = [np.ascontiguousarray(np.asarray(Wk)[:, t * DL : (t + 1) * DL]).astype(bf) for t in range(TP)]
    wvs = [np.ascontiguousarray(np.asarray(Wv)[:, t * DL : (t + 1) * DL]).astype(bf) for t in range(TP)]
    wos = [np.ascontiguousarray(np.asarray(Wo)[t * DL : (t + 1) * DL, :]).astype(bf) for t in range(TP)]

    in_maps = []
    for b in range(B):
        for t in range(TP):
            in_maps.append(
                {
                    "xt": xtb[b],
                    "yt": ytb[b],
                    "wq": wqs[t],
                    "wk": wks[t],
                    "wv": wvs[t],
                    "wo": wos[t],
                }
            )

    nc = _get_nc()
    res = run_bass_kernel_spmd(
        nc, in_maps, core_ids=list(range(N_CORES)), trace=_trace
    )
    out = np.empty((B, L, U), np.float32)
    for b in range(B):
        out[b] = (
            np.asarray(res.results[2 * b]["out_a"], np.float32)
            + np.asarray(res.results[2 * b]["out_c"], np.float32)
            + np.asarray(res.results[2 * b + 1]["out_a"], np.float32)
            + np.asarray(res.results[2 * b + 1]["out_c"], np.float32)
        )
    if _trace:
        return out, res
    return out

